# revision 53
# baseline (speedup 1.0000x reference)
"""Trainium2 Bass kernel for nn_Loss_60567628808292 (YOLO-style loss).

Strategy (8 NeuronCores, data-parallel on batch):
  * noobj confidence term (the memory-bound bulk): each core streams its
    2048-batch shard as 4 chunks of an interleaved [P, 2*f] plane
    (pred-half | target-half per partition row -> one ~54KB descriptor
    per partition, 256B-aligned); chunk 1 rides the ACT HWDGE ring so the
    SP ring's serialization gap overlaps.
    Conf channels 4/9 are extracted with strided SBUF views; mask-mul +
    square + reduce fold into 4 DVE ops per chunk (the square+reduce via
    scalar_tensor_tensor's per-partition accumulator).  The LAST chunk's
    contribution computes early from a dense conf replica appended to the
    pfx transfer, so nothing on the post-stream critical path waits for
    compute -- the tail is just DMA receipt + the fixed epilogue.
  * bbox term: the reference truncates at global rank < 49 (= S*S) object
    cells; at ~25% object density the 49th object cell sits near flat
    index ~200, so a 1024-cell prefix (PFXF=8 cells x 128 partitions) has
    5x margin.  make_inputs verifies the actual cutoff and falls back to
    a wider prefix build if ever needed.  The pfx DMA rides the ACT HWDGE
    ring ahead of the stream so the bbox math overlaps the stream fully;
    every core computes it redundantly (SPMD), core 0's value is used.
  * host sums the tiny [128,2] per-core partials (the scalar all-reduce).

Measured constraints that shaped this (don't regress them):
  * stream DMAs must cover all 128 partitions and be 256B-aligned in
    DRAM (chunk cells % 16 == 0): sub-128-row transfers run at ~250 GB/s
    vs ~455 GB/s, misaligned rows at ~175 GB/s, on both HWDGE and SWDGE.
  * one SDMA engine (always the first or last of the core's 16) is
    intermittently ~20% slow on some devices; descriptor dealing is
    round-robin per transfer so its 1/16 share caps an afflicted core at
    ~90us vs ~80us healthy.  Engine-targeted load skew is impossible at
    line rate (needs sub-128-row transfers).
  * ~8.5us NEFF startup and a ~12us fixed walrus semaphore-reset
    postamble bound the total from below regardless of program size.
"""

import numpy as np

import concourse.bass as bass
import concourse.tile as tile
from concourse import mybir
from concourse.bass_utils import run_bass_kernel_spmd

# problem constants (hardcoded per spec)
S = 7.0
NCORES = 8
BATCH = 16384
CELLS = 49           # 7*7
N = 30
P = 128
SHARD_B = BATCH // NCORES              # 2048
SHARD_CELLS = SHARD_B * CELLS          # 100_352 cells per core
# NOTE (measured): stream transfers must cover all 128 partitions (sub-128
# transfers run at ~55% rate) and chunk cell counts must be multiples of 16
# (16 cells * 240B = 3840B keeps DMA descriptor sources 256B-aligned -- the
# DRAM page size; misaligned rows cost ~2.4x bandwidth).
P_ACT = 128                            # partitions carrying stream data
CELLS_PP = SHARD_CELLS // P_ACT        # 784 cells per partition
# uneven chunk cell counts (per partition): smaller last chunk shortens the
# post-stream compute tail.  Exactly 4 chunks: every extra transfer
# boundary stretches the stream ~1.3-1.5us (measured), so 5-chunk splits
# lose despite a smaller tail.
CHUNK_CELLS = (224, 224, 224, 112)
# sqrt(v) on v in [0.178, 0.485] via cubic; the constant term cancels in
# sqrt-differences and the leading coeff folds into the l2 sum:
# q(v) = ((v + A)*v + B)*v, scaled by SC = c3^2 (l2-term rel err ~8e-5).
# This keeps the ACT engine compute-free so the bulk stream can ride its
# HWDGE ring (q10): the intermittent per-device slow SDMA engine serves
# q1 at ~14 GB/s but q10 at full rate, so q10-streaming dodges the tax.
SQ_A = -1.7067714662597755
SQ_B = 1.677284565416749
SQ_SC = 0.9862877069623385
assert sum(CHUNK_CELLS) == CELLS_PP
W = 2 * CELLS_PP * N                   # 47040 floats per partition row
PFXF = 8                               # prefix cells per partition (1024 total)
L_NOOBJ = 0.5

_A = mybir.AluOpType
_f32 = mybir.dt.float32


def build_nc(chunks=CHUNK_CELLS, pfxf=PFXF, hoist=True):
    G = 4 * pfxf          # one channel plane: [pred b0, pred b1, tgt b0, tgt b1]
    PH = 2 * pfxf         # a box pair (b0, b1)
    # pfx layout per partition: A(2G: x|y) B(2G: w|h) C(G: conf) M(pfxf),
    # then a dense (p4,p9)/(t4,t9) replica of the LAST chunk's conf channels:
    # its noobj contribution computes from this early-arriving copy, so the
    # post-stream critical path is just DMA-receipt + the fixed epilogue
    # (the last chunk still streams in full; its tile is simply not read).
    cl = chunks[-1]
    PFXW = 5 * G + pfxf + 4 * cl
    nchunk = len(chunks)
    maxc = max(chunks)

    nc = bass.Bass()
    xy = nc.declare_dram_parameter("xy", [P, W], _f32, isOutput=False)
    pfx = nc.declare_dram_parameter("pfx", [P, PFXW], _f32, isOutput=False)
    # out cols: [noobj chunks 0..n-2 (reduced mid-stream), bbox, noobj last
    # chunk (written by its accumulator directly -- keeps the final reduce
    # off the post-stream critical path)]
    out = nc.declare_dram_parameter("out", [P, 3], _f32, isOutput=True)

    io_bufs = 3 if pfxf <= 32 else 2  # wide-prefix fallback: fit SBUF
    with tile.TileContext(nc) as tc:
        with (
            tc.tile_pool(name="io", bufs=io_bufs) as io,
            tc.tile_pool(name="tp", bufs=2) as tp,
            tc.tile_pool(name="bb", bufs=1) as bb,
            tc.tile_pool(name="accp", bufs=1) as accp,
        ):
            V = nc.vector
            acc = accp.tile([P, nchunk - 1], _f32)
            res = accp.tile([P, 3], _f32)

            # pfx rides the SP ring; the bulk stream rides the ACT ring
            # (q10) to dodge the q1-specific slow-engine tax.  The ACT
            # engine carries no compute (DVE sqrt above) so its DMA issues
            # never block.
            pt = bb.tile([P, PFXW], _f32)
            nc.sync.dma_start(out=pt[:], in_=pfx[:])

            cts = []
            off = 0
            rings = (nc.scalar, nc.sync, nc.scalar, nc.scalar)
            for i, c in enumerate(chunks):
                w = 2 * c * N
                ct = io.tile([P, 2 * maxc * N], _f32, tag="ct")
                rings[i].dma_start(out=ct[:, 0:w], in_=xy[:, off:off + w])
                cts.append(ct)
                off += 2 * c * N

            # ---------------- bbox prefix (overlaps the stream) ----------
            A = pt[:, 0:2 * G]
            B = pt[:, 2 * G:4 * G]
            C = pt[:, 4 * G:5 * G]
            M = pt[:, 5 * G:5 * G + pfxf]

            def t2(name, w):
                return bb.tile([P, w], _f32, tag=name, name=name)

            hB = t2("hB", 2 * G)
            V.tensor_scalar_mul(hB[:], B, 0.5)
            XY1 = t2("XY1", 2 * G)
            V.scalar_tensor_tensor(XY1[:], A, 1.0 / S, hB[:], _A.mult, _A.subtract)
            XY2 = t2("XY2", 2 * G)
            V.scalar_tensor_tensor(XY2[:], XY1[:], 1.0 / S, hB[:], _A.mult, _A.add)
            SXY2 = t2("SXY2", 2 * G)  # DVE cubic (constant term omitted --
            # cancels in the tgt-pred difference below)
            V.scalar_tensor_tensor(SXY2[:], XY2[:], SQ_A, XY2[:], _A.add, _A.mult)
            V.scalar_tensor_tensor(SXY2[:], SXY2[:], SQ_B, XY2[:], _A.add, _A.mult)

            def predv(t):  # [2(xy), 2*pfxf] pred-box pair of both xy halves
                return t[:].rearrange("p (c q) -> p c q", c=2)[:, :, 0:PH]

            def tgtv(t):
                return t[:].rearrange("p (c q) -> p c q", c=2)[:, :, PH:2 * PH]

            D1 = t2("D1", 2 * PH)
            V.tensor_sub(D1[:].rearrange("p (c q) -> p c q", c=2), tgtv(XY1), predv(XY1))
            D2 = t2("D2", 2 * PH)
            V.tensor_sub(D2[:].rearrange("p (c q) -> p c q", c=2), tgtv(SXY2), predv(SXY2))
            S1 = t2("S1", 2 * PH)
            V.tensor_mul(S1[:], D1[:], D1[:])
            S2 = t2("S2", 2 * PH)
            V.tensor_mul(S2[:], D2[:], D2[:])
            T12 = t2("T12", 2 * PH)  # S1 + c3^2 * S2 (sqrt-poly leading coeff)
            V.scalar_tensor_tensor(T12[:], S2[:], SQ_SC, S1[:], _A.mult, _A.add)
            l12 = t2("l12", PH)  # 5*(dx^2+ex^2) + (dy^2+ey^2)
            V.scalar_tensor_tensor(l12[:], T12[:, 0:PH], 5.0, T12[:, PH:2 * PH], _A.mult, _A.add)

            d3 = t2("d3", PH)
            V.tensor_sub(d3[:], C[:, PH:2 * PH], C[:, 0:PH])
            S3 = t2("S3", PH)
            V.tensor_mul(S3[:], d3[:], d3[:])

            LT = t2("LT", 2 * PH)
            V.tensor_max(LT[:].rearrange("p (c q) -> p c q", c=2), predv(XY1), tgtv(XY1))
            RB = t2("RB", 2 * PH)
            V.tensor_tensor(RB[:].rearrange("p (c q) -> p c q", c=2), predv(XY2), tgtv(XY2), _A.min)
            WHt = t2("WHt", 2 * PH)
            V.tensor_sub(WHt[:], RB[:], LT[:])
            V.tensor_single_scalar(WHt[:], WHt[:], 0.0, _A.max)
            inter = t2("inter", PH)
            V.tensor_mul(inter[:], WHt[:, 0:PH], WHt[:, PH:2 * PH])
            AWH = t2("AWH", 2 * G)
            V.tensor_sub(AWH[:], XY2[:], XY1[:])
            area = t2("area", G)
            V.tensor_mul(area[:], AWH[:, 0:G], AWH[:, G:2 * G])
            uni = t2("uni", PH)
            V.tensor_add(uni[:], area[:, 0:PH], area[:, PH:2 * PH])
            V.tensor_sub(uni[:], uni[:], inter[:])
            V.reciprocal(uni[:], uni[:])
            iou = t2("iou", PH)
            V.tensor_mul(iou[:], inter[:], uni[:])

            tot = t2("tot", PH)
            V.tensor_add(tot[:], l12[:], S3[:])
            V.tensor_add(tot[:], tot[:], iou[:])
            jm = bb.tile([P, pfxf], mybir.dt.uint8, tag="jm")
            V.tensor_tensor(jm[:], iou[:, pfxf:PH], iou[:, 0:pfxf], _A.is_gt)
            sel = t2("sel", pfxf)
            V.tensor_copy(sel[:], tot[:, 0:pfxf])
            V.copy_predicated(sel[:], jm[:], tot[:, pfxf:PH])
            dump = t2("dump", pfxf)
            V.scalar_tensor_tensor(
                dump[:], sel[:], 1.0, M, _A.mult, _A.mult,
                accum_out=res[:, 1:2],
            )

            # ---- last chunk's noobj from the dense pfx replica (early) ----
            ds0 = 5 * G + pfxf
            xsd = pt[:, ds0:ds0 + 2 * cl].rearrange("p (n k) -> p n k", k=2)
            tsd = pt[:, ds0 + 2 * cl:ds0 + 4 * cl].rearrange("p (n k) -> p n k", k=2)
            ml = t2("ml", cl)
            V.tensor_single_scalar(ml[:], tsd[:, :, 0], 0.0, _A.is_le)
            dl = t2("dl", 2 * cl)
            V.tensor_sub(dl[:].rearrange("p (n k) -> p n k", k=2), xsd, tsd)
            dml = t2("dml", 2 * cl)
            mlb = ml[:].unsqueeze(2).broadcast_to((P, cl, 2))
            V.tensor_mul(dml[:].rearrange("p (n k) -> p n k", k=2),
                         dl[:].rearrange("p (n k) -> p n k", k=2), mlb)
            scl = t2("scl", 2 * cl)
            V.scalar_tensor_tensor(
                scl[:], dml[:], 1.0, dml[:], _A.mult, _A.mult,
                accum_out=res[:, 2:3],
            )

            # ---------------- noobj stream compute (chunks 0..n-2) --------
            PA = P_ACT
            for i, c in enumerate(chunks[:-1]):
                ct = cts[i]
                f = c * N
                # [p, cell, a(6), b(5)]: channel 4 = (a0,b4), channel 9 = (a1,b4)
                xv = ct[0:PA, 0:f].rearrange("p (n a b) -> p n a b", a=6, b=5)
                yv = ct[0:PA, f:2 * f].rearrange("p (n a b) -> p n a b", a=6, b=5)
                xconf = xv[:, :, 0:2, 4]     # [pa, c, 2]
                yconf = yv[:, :, 0:2, 4]
                t4 = yv[:, :, 0, 4]          # [pa, c]

                m = tp.tile([P, maxc], _f32, tag="m")
                V.tensor_single_scalar(m[0:PA, 0:c], t4, 0.0, _A.is_le)
                d = tp.tile([P, 2 * maxc], _f32, tag="d")
                dv = d[0:PA, 0:2 * c].rearrange("p (n k) -> p n k", k=2)
                V.tensor_sub(dv, xconf, yconf)
                dm = tp.tile([P, 2 * maxc], _f32, tag="dm")
                mb = m[0:PA, 0:c].unsqueeze(2).broadcast_to((PA, c, 2))
                V.tensor_mul(dm[0:PA, 0:2 * c].rearrange("p (n k) -> p n k", k=2), dv, mb)
                scr = tp.tile([P, 2 * maxc], _f32, tag="scr")
                V.scalar_tensor_tensor(
                    scr[0:PA, 0:2 * c], dm[0:PA, 0:2 * c], 1.0, dm[0:PA, 0:2 * c],
                    _A.mult, _A.mult, accum_out=acc[0:PA, i:i + 1],
                )
                if i == nchunk - 2:
                    V.reduce_sum(res[:, 0:1], acc[:], axis=mybir.AxisListType.X)

            nc.sync.dma_start(out=out[:], in_=res[:])

    if hoist:  # required by the walrus HW build; current CoreSim rejects it
        _split_multi_waits(nc)
    return nc


def _split_multi_waits(nc):
    """This walrus build allows only one attached sync-wait per instruction;
    hoist extras into standalone event-semaphore waits (engines are in-order,
    so a preceding wait instruction on the same engine is equivalent)."""
    f = nc.m.functions[0]
    for blk in f.blocks:
        new = []
        changed = False
        for ins in blk.instructions:
            si = ins.sync_info
            ow = list(si.on_wait) if (si is not None and si.on_wait) else []
            if len(ow) > 1:
                for k, w in enumerate(ow):
                    ev = mybir.InstEventSemaphore(
                        name=f"{ins.name}_hw{k}", ins=[], outs=[],
                        sync_info=mybir.SyncInfo(on_wait=[w], on_update=[]),
                    )
                    ev.engine = ins.engine
                    new.append(ev)
                ins.sync_info = mybir.SyncInfo(
                    on_wait=[], on_update=list(si.on_update)
                )
                changed = True
            new.append(ins)
        if changed:
            blk.instructions = new


def make_inputs(pred, target):
    """Full inputs -> (in_maps list of 8 per-core dicts, pfxf used)."""
    pred = np.ascontiguousarray(np.asarray(pred, dtype=np.float32))
    target = np.ascontiguousarray(np.asarray(target, dtype=np.float32))
    # pad each core's cells up to P_ACT x CELLS_PP if needed (zero cells
    # contribute 0 to the masked noobj sum); currently an exact fit (npad=0)
    npad = P_ACT * CELLS_PP - SHARD_CELLS
    xs = np.concatenate(
        [pred.reshape(NCORES, SHARD_CELLS, N),
         np.zeros((NCORES, npad, N), np.float32)], axis=1
    ).reshape(NCORES, P_ACT, CELLS_PP, N)
    ys = np.concatenate(
        [target.reshape(NCORES, SHARD_CELLS, N),
         np.zeros((NCORES, npad, N), np.float32)], axis=1
    ).reshape(NCORES, P_ACT, CELLS_PP, N)
    blocks = []
    a = 0
    for c in CHUNK_CELLS:
        blocks.append(xs[:, :, a:a + c].reshape(NCORES, P_ACT, c * N))
        blocks.append(ys[:, :, a:a + c].reshape(NCORES, P_ACT, c * N))
        a += c
    xyf = np.zeros((NCORES, P, W), np.float32)
    xyf[:, 0:P_ACT] = np.concatenate(blocks, axis=2)

    # global object ranks (from target conf ch4) -> active mask
    t4all = target.reshape(-1, N)[:, 4]
    obj = t4all > 0
    rank = np.cumsum(obj.astype(np.int64)) - 1
    active = obj & (rank < CELLS)
    last_active = int(np.max(np.nonzero(active)[0])) if active.any() else -1

    pfxf = PFXF
    while P * pfxf <= last_active:  # fallback: widen prefix (never for sane inputs)
        pfxf *= 4

    npfx = P * pfxf
    pp = pred.reshape(-1, N)[:npfx]
    tt = target.reshape(-1, N)[:npfx]
    # channel plane [4, npfx]: pred b0, pred b1, tgt b0, tgt b1
    ch = np.empty((5, 4, npfx), np.float32)
    for ci in range(5):  # x, y, w, h, conf
        ch[ci, 0] = pp[:, ci]
        ch[ci, 1] = pp[:, ci + 5]
        ch[ci, 2] = tt[:, ci]
        ch[ci, 3] = tt[:, ci + 5]
    # -> per-partition plane blocks [.., P, pfxf] -> [P, ..]
    chp = ch.reshape(5, 4, P, pfxf).transpose(2, 0, 1, 3)  # [P, 5, 4, pfxf]
    Ap = chp[:, 0:2]                      # [P, 2(xy), 4, pfxf]
    Bp = chp[:, 2:4]                      # [P, 2(wh), 4, pfxf]
    Cp = chp[:, 4]                        # [P, 4, pfxf]
    Mp = active[:npfx].astype(np.float32).reshape(P, pfxf)
    common = np.concatenate(
        [Ap.reshape(P, -1), Bp.reshape(P, -1), Cp.reshape(P, -1), Mp], axis=1)

    # per-core dense (p4,p9)/(t4,t9) replica of the LAST chunk's cells
    cl = CHUNK_CELLS[-1]
    a0 = CELLS_PP - cl
    lastx = xs[:, :, a0:, :][:, :, :, [4, 9]].reshape(NCORES, P, 2 * cl)
    lastt = ys[:, :, a0:, :][:, :, :, [4, 9]].reshape(NCORES, P, 2 * cl)
    return [
        {"xy": xyf[c],
         "pfx": np.ascontiguousarray(
             np.concatenate([common, lastx[c], lastt[c]], axis=1))}
        for c in range(NCORES)
    ], pfxf


def reduce_outputs(outs):
    """Per-core {"out": [128,3]} results -> scalar loss."""
    noobj = sum(
        o["out"][:, 0].astype(np.float64).sum() + o["out"][:, 2].astype(np.float64).sum()
        for o in outs
    )
    bbox = outs[0]["out"][:, 1].astype(np.float64).sum()
    return np.float32(L_NOOBJ * noobj + bbox)


_NC_CACHE = {}


def _get_nc(pfxf):
    if pfxf not in _NC_CACHE:
        _NC_CACHE[pfxf] = build_nc(pfxf=pfxf)
    return _NC_CACHE[pfxf]


def run(pred, target, **spmd_kwargs):
    in_maps, pfxf = make_inputs(pred, target)
    nc = _get_nc(pfxf)
    res = run_bass_kernel_spmd(nc, in_maps, list(range(NCORES)), **spmd_kwargs)
    return reduce_outputs(res.results), res


def kernel(pred, target):
    val, _ = run(pred, target)
    return val


# revision 54
# speedup vs baseline: 1.0261x; 1.0261x over previous
"""Trainium2 Bass kernel for nn_Loss_60567628808292 (YOLO-style loss).

Strategy (8 NeuronCores, data-parallel on batch):
  * noobj confidence term (the memory-bound bulk): each core streams its
    2048-batch shard as 4 chunks of an interleaved [P, 2*f] plane
    (pred-half | target-half per partition row -> one ~54KB descriptor
    per partition, 256B-aligned); chunk 1 rides the ACT HWDGE ring so the
    SP ring's serialization gap overlaps.
    Conf channels 4/9 are extracted with strided SBUF views; mask-mul +
    square + reduce fold into 4 DVE ops per chunk (the square+reduce via
    scalar_tensor_tensor's per-partition accumulator).  The LAST chunk's
    contribution computes early from a dense conf replica appended to the
    pfx transfer, so nothing on the post-stream critical path waits for
    compute -- the tail is just DMA receipt + the fixed epilogue.
  * bbox term: the reference truncates at global rank < 49 (= S*S) object
    cells; at ~25% object density the 49th object cell sits near flat
    index ~200, so a 1024-cell prefix (PFXF=8 cells x 128 partitions) has
    5x margin.  make_inputs verifies the actual cutoff and falls back to
    a wider prefix build if ever needed.  The pfx DMA rides the ACT HWDGE
    ring ahead of the stream so the bbox math overlaps the stream fully;
    every core computes it redundantly (SPMD), core 0's value is used.
  * host sums the tiny [128,2] per-core partials (the scalar all-reduce).

Measured constraints that shaped this (don't regress them):
  * stream DMAs must cover all 128 partitions and be 256B-aligned in
    DRAM (chunk cells % 16 == 0): sub-128-row transfers run at ~250 GB/s
    vs ~455 GB/s, misaligned rows at ~175 GB/s, on both HWDGE and SWDGE.
  * one SDMA engine (always the first or last of the core's 16) is
    intermittently ~20% slow on some devices; descriptor dealing is
    round-robin per transfer so its 1/16 share caps an afflicted core at
    ~90us vs ~80us healthy.  Engine-targeted load skew is impossible at
    line rate (needs sub-128-row transfers).
  * ~8.5us NEFF startup and a ~12us fixed walrus semaphore-reset
    postamble bound the total from below regardless of program size.
"""

import numpy as np

import concourse.bass as bass
import concourse.tile as tile
from concourse import mybir
from concourse.bass_utils import run_bass_kernel_spmd

# problem constants (hardcoded per spec)
S = 7.0
NCORES = 8
BATCH = 16384
CELLS = 49           # 7*7
N = 30
P = 128
SHARD_B = BATCH // NCORES              # 2048
SHARD_CELLS = SHARD_B * CELLS          # 100_352 cells per core
# NOTE (measured): stream transfers must cover all 128 partitions (sub-128
# transfers run at ~55% rate) and chunk cell counts must be multiples of 16
# (16 cells * 240B = 3840B keeps DMA descriptor sources 256B-aligned -- the
# DRAM page size; misaligned rows cost ~2.4x bandwidth).
P_ACT = 128                            # partitions carrying stream data
CELLS_PP = SHARD_CELLS // P_ACT        # 784 cells per partition
# uneven chunk cell counts (per partition): smaller last chunk shortens the
# post-stream compute tail.  Exactly 4 chunks: every extra transfer
# boundary stretches the stream ~1.3-1.5us (measured), so 5-chunk splits
# lose despite a smaller tail.
CHUNK_CELLS = (224, 224, 224, 112)
# (Tried: DVE cubic sqrt to free the ACT ring + streaming on q10 to
# dodge the slow-engine tax -- the straggler FOLLOWS the load to q10
# (it is load-following arbitration, not queue-specific), so reverted.)
assert sum(CHUNK_CELLS) == CELLS_PP
W = 2 * CELLS_PP * N                   # 47040 floats per partition row
PFXF = 8                               # prefix cells per partition (1024 total)
L_NOOBJ = 0.5

_A = mybir.AluOpType
_f32 = mybir.dt.float32


def build_nc(chunks=CHUNK_CELLS, pfxf=PFXF, hoist=True):
    G = 4 * pfxf          # one channel plane: [pred b0, pred b1, tgt b0, tgt b1]
    PH = 2 * pfxf         # a box pair (b0, b1)
    # pfx layout per partition: A(2G: x|y) B(2G: w|h) C(G: conf) M(pfxf),
    # then a dense (p4,p9)/(t4,t9) replica of the LAST chunk's conf channels:
    # its noobj contribution computes from this early-arriving copy, so the
    # post-stream critical path is just DMA-receipt + the fixed epilogue
    # (the last chunk still streams in full; its tile is simply not read).
    cl = chunks[-1]
    PFXW = 5 * G + pfxf + 4 * cl
    nchunk = len(chunks)
    maxc = max(chunks)

    nc = bass.Bass()
    xy = nc.declare_dram_parameter("xy", [P, W], _f32, isOutput=False)
    pfx = nc.declare_dram_parameter("pfx", [P, PFXW], _f32, isOutput=False)
    # out cols: [noobj chunks 0..n-2 (reduced mid-stream), bbox, noobj last
    # chunk (written by its accumulator directly -- keeps the final reduce
    # off the post-stream critical path)]
    out = nc.declare_dram_parameter("out", [P, 3], _f32, isOutput=True)

    io_bufs = 3 if pfxf <= 32 else 2  # wide-prefix fallback: fit SBUF
    with tile.TileContext(nc) as tc:
        with (
            tc.tile_pool(name="io", bufs=io_bufs) as io,
            tc.tile_pool(name="tp", bufs=2) as tp,
            tc.tile_pool(name="bb", bufs=1) as bb,
            tc.tile_pool(name="accp", bufs=1) as accp,
        ):
            V = nc.vector
            acc = accp.tile([P, nchunk - 1], _f32)
            res = accp.tile([P, 3], _f32)

            # pfx rides the ACT HWDGE ring ahead of its first stream chunk;
            # chunk 1 also rides ACT so the SP ring's serialization gap
            # overlaps.  Later chunks must NOT use ACT -- the in-order ACT
            # queue parks their issue behind the bbox sqrt (ACTIVATE).
            pt = bb.tile([P, PFXW], _f32)
            nc.scalar.dma_start(out=pt[:], in_=pfx[:])

            cts = []
            off = 0
            rings = (nc.sync, nc.scalar, nc.sync, nc.sync)
            for i, c in enumerate(chunks):
                w = 2 * c * N
                ct = io.tile([P, 2 * maxc * N], _f32, tag="ct")
                rings[i].dma_start(out=ct[:, 0:w], in_=xy[:, off:off + w])
                cts.append(ct)
                off += 2 * c * N

            # ---------------- bbox prefix (overlaps the stream) ----------
            A = pt[:, 0:2 * G]
            B = pt[:, 2 * G:4 * G]
            C = pt[:, 4 * G:5 * G]
            M = pt[:, 5 * G:5 * G + pfxf]

            def t2(name, w):
                return bb.tile([P, w], _f32, tag=name, name=name)

            hB = t2("hB", 2 * G)
            V.tensor_scalar_mul(hB[:], B, 0.5)
            XY1 = t2("XY1", 2 * G)
            V.scalar_tensor_tensor(XY1[:], A, 1.0 / S, hB[:], _A.mult, _A.subtract)
            XY2 = t2("XY2", 2 * G)
            V.scalar_tensor_tensor(XY2[:], XY1[:], 1.0 / S, hB[:], _A.mult, _A.add)
            SXY2 = t2("SXY2", 2 * G)
            nc.scalar.sqrt(SXY2[:], XY2[:])

            def predv(t):  # [2(xy), 2*pfxf] pred-box pair of both xy halves
                return t[:].rearrange("p (c q) -> p c q", c=2)[:, :, 0:PH]

            def tgtv(t):
                return t[:].rearrange("p (c q) -> p c q", c=2)[:, :, PH:2 * PH]

            D1 = t2("D1", 2 * PH)
            V.tensor_sub(D1[:].rearrange("p (c q) -> p c q", c=2), tgtv(XY1), predv(XY1))
            D2 = t2("D2", 2 * PH)
            V.tensor_sub(D2[:].rearrange("p (c q) -> p c q", c=2), tgtv(SXY2), predv(SXY2))
            S1 = t2("S1", 2 * PH)
            V.tensor_mul(S1[:], D1[:], D1[:])
            S2 = t2("S2", 2 * PH)
            V.tensor_mul(S2[:], D2[:], D2[:])
            T12 = t2("T12", 2 * PH)
            V.tensor_add(T12[:], S1[:], S2[:])
            l12 = t2("l12", PH)  # 5*(dx^2+ex^2) + (dy^2+ey^2)
            V.scalar_tensor_tensor(l12[:], T12[:, 0:PH], 5.0, T12[:, PH:2 * PH], _A.mult, _A.add)

            d3 = t2("d3", PH)
            V.tensor_sub(d3[:], C[:, PH:2 * PH], C[:, 0:PH])
            S3 = t2("S3", PH)
            V.tensor_mul(S3[:], d3[:], d3[:])

            LT = t2("LT", 2 * PH)
            V.tensor_max(LT[:].rearrange("p (c q) -> p c q", c=2), predv(XY1), tgtv(XY1))
            RB = t2("RB", 2 * PH)
            V.tensor_tensor(RB[:].rearrange("p (c q) -> p c q", c=2), predv(XY2), tgtv(XY2), _A.min)
            WHt = t2("WHt", 2 * PH)
            V.tensor_sub(WHt[:], RB[:], LT[:])
            V.tensor_single_scalar(WHt[:], WHt[:], 0.0, _A.max)
            inter = t2("inter", PH)
            V.tensor_mul(inter[:], WHt[:, 0:PH], WHt[:, PH:2 * PH])
            AWH = t2("AWH", 2 * G)
            V.tensor_sub(AWH[:], XY2[:], XY1[:])
            area = t2("area", G)
            V.tensor_mul(area[:], AWH[:, 0:G], AWH[:, G:2 * G])
            uni = t2("uni", PH)
            V.tensor_add(uni[:], area[:, 0:PH], area[:, PH:2 * PH])
            V.tensor_sub(uni[:], uni[:], inter[:])
            V.reciprocal(uni[:], uni[:])
            iou = t2("iou", PH)
            V.tensor_mul(iou[:], inter[:], uni[:])

            tot = t2("tot", PH)
            V.tensor_add(tot[:], l12[:], S3[:])
            V.tensor_add(tot[:], tot[:], iou[:])
            jm = bb.tile([P, pfxf], mybir.dt.uint8, tag="jm")
            V.tensor_tensor(jm[:], iou[:, pfxf:PH], iou[:, 0:pfxf], _A.is_gt)
            sel = t2("sel", pfxf)
            V.tensor_copy(sel[:], tot[:, 0:pfxf])
            V.copy_predicated(sel[:], jm[:], tot[:, pfxf:PH])
            dump = t2("dump", pfxf)
            V.scalar_tensor_tensor(
                dump[:], sel[:], 1.0, M, _A.mult, _A.mult,
                accum_out=res[:, 1:2],
            )

            # ---- last chunk's noobj from the dense pfx replica (early) ----
            ds0 = 5 * G + pfxf
            xsd = pt[:, ds0:ds0 + 2 * cl].rearrange("p (n k) -> p n k", k=2)
            tsd = pt[:, ds0 + 2 * cl:ds0 + 4 * cl].rearrange("p (n k) -> p n k", k=2)
            ml = t2("ml", cl)
            V.tensor_single_scalar(ml[:], tsd[:, :, 0], 0.0, _A.is_le)
            dl = t2("dl", 2 * cl)
            V.tensor_sub(dl[:].rearrange("p (n k) -> p n k", k=2), xsd, tsd)
            dml = t2("dml", 2 * cl)
            mlb = ml[:].unsqueeze(2).broadcast_to((P, cl, 2))
            V.tensor_mul(dml[:].rearrange("p (n k) -> p n k", k=2),
                         dl[:].rearrange("p (n k) -> p n k", k=2), mlb)
            scl = t2("scl", 2 * cl)
            V.scalar_tensor_tensor(
                scl[:], dml[:], 1.0, dml[:], _A.mult, _A.mult,
                accum_out=res[:, 2:3],
            )

            # ---------------- noobj stream compute (chunks 0..n-2) --------
            PA = P_ACT
            for i, c in enumerate(chunks[:-1]):
                ct = cts[i]
                f = c * N
                # [p, cell, a(6), b(5)]: channel 4 = (a0,b4), channel 9 = (a1,b4)
                xv = ct[0:PA, 0:f].rearrange("p (n a b) -> p n a b", a=6, b=5)
                yv = ct[0:PA, f:2 * f].rearrange("p (n a b) -> p n a b", a=6, b=5)
                xconf = xv[:, :, 0:2, 4]     # [pa, c, 2]
                yconf = yv[:, :, 0:2, 4]
                t4 = yv[:, :, 0, 4]          # [pa, c]

                m = tp.tile([P, maxc], _f32, tag="m")
                V.tensor_single_scalar(m[0:PA, 0:c], t4, 0.0, _A.is_le)
                d = tp.tile([P, 2 * maxc], _f32, tag="d")
                dv = d[0:PA, 0:2 * c].rearrange("p (n k) -> p n k", k=2)
                V.tensor_sub(dv, xconf, yconf)
                dm = tp.tile([P, 2 * maxc], _f32, tag="dm")
                mb = m[0:PA, 0:c].unsqueeze(2).broadcast_to((PA, c, 2))
                V.tensor_mul(dm[0:PA, 0:2 * c].rearrange("p (n k) -> p n k", k=2), dv, mb)
                scr = tp.tile([P, 2 * maxc], _f32, tag="scr")
                V.scalar_tensor_tensor(
                    scr[0:PA, 0:2 * c], dm[0:PA, 0:2 * c], 1.0, dm[0:PA, 0:2 * c],
                    _A.mult, _A.mult, accum_out=acc[0:PA, i:i + 1],
                )
                if i == nchunk - 2:
                    V.reduce_sum(res[:, 0:1], acc[:], axis=mybir.AxisListType.X)

            nc.scalar.dma_start(out=out[:], in_=res[:])

    if hoist:  # required by the walrus HW build; current CoreSim rejects it
        _split_multi_waits(nc)
    return nc


def _split_multi_waits(nc):
    """This walrus build allows only one attached sync-wait per instruction;
    hoist extras into standalone event-semaphore waits (engines are in-order,
    so a preceding wait instruction on the same engine is equivalent)."""
    f = nc.m.functions[0]
    for blk in f.blocks:
        new = []
        changed = False
        for ins in blk.instructions:
            si = ins.sync_info
            ow = list(si.on_wait) if (si is not None and si.on_wait) else []
            if len(ow) > 1:
                for k, w in enumerate(ow):
                    ev = mybir.InstEventSemaphore(
                        name=f"{ins.name}_hw{k}", ins=[], outs=[],
                        sync_info=mybir.SyncInfo(on_wait=[w], on_update=[]),
                    )
                    ev.engine = ins.engine
                    new.append(ev)
                ins.sync_info = mybir.SyncInfo(
                    on_wait=[], on_update=list(si.on_update)
                )
                changed = True
            new.append(ins)
        if changed:
            blk.instructions = new


def make_inputs(pred, target):
    """Full inputs -> (in_maps list of 8 per-core dicts, pfxf used)."""
    pred = np.ascontiguousarray(np.asarray(pred, dtype=np.float32))
    target = np.ascontiguousarray(np.asarray(target, dtype=np.float32))
    # pad each core's cells up to P_ACT x CELLS_PP if needed (zero cells
    # contribute 0 to the masked noobj sum); currently an exact fit (npad=0)
    npad = P_ACT * CELLS_PP - SHARD_CELLS
    xs = np.concatenate(
        [pred.reshape(NCORES, SHARD_CELLS, N),
         np.zeros((NCORES, npad, N), np.float32)], axis=1
    ).reshape(NCORES, P_ACT, CELLS_PP, N)
    ys = np.concatenate(
        [target.reshape(NCORES, SHARD_CELLS, N),
         np.zeros((NCORES, npad, N), np.float32)], axis=1
    ).reshape(NCORES, P_ACT, CELLS_PP, N)
    blocks = []
    a = 0
    for c in CHUNK_CELLS:
        blocks.append(xs[:, :, a:a + c].reshape(NCORES, P_ACT, c * N))
        blocks.append(ys[:, :, a:a + c].reshape(NCORES, P_ACT, c * N))
        a += c
    xyf = np.zeros((NCORES, P, W), np.float32)
    xyf[:, 0:P_ACT] = np.concatenate(blocks, axis=2)

    # global object ranks (from target conf ch4) -> active mask
    t4all = target.reshape(-1, N)[:, 4]
    obj = t4all > 0
    rank = np.cumsum(obj.astype(np.int64)) - 1
    active = obj & (rank < CELLS)
    last_active = int(np.max(np.nonzero(active)[0])) if active.any() else -1

    pfxf = PFXF
    while P * pfxf <= last_active:  # fallback: widen prefix (never for sane inputs)
        pfxf *= 4

    npfx = P * pfxf
    pp = pred.reshape(-1, N)[:npfx]
    tt = target.reshape(-1, N)[:npfx]
    # channel plane [4, npfx]: pred b0, pred b1, tgt b0, tgt b1
    ch = np.empty((5, 4, npfx), np.float32)
    for ci in range(5):  # x, y, w, h, conf
        ch[ci, 0] = pp[:, ci]
        ch[ci, 1] = pp[:, ci + 5]
        ch[ci, 2] = tt[:, ci]
        ch[ci, 3] = tt[:, ci + 5]
    # -> per-partition plane blocks [.., P, pfxf] -> [P, ..]
    chp = ch.reshape(5, 4, P, pfxf).transpose(2, 0, 1, 3)  # [P, 5, 4, pfxf]
    Ap = chp[:, 0:2]                      # [P, 2(xy), 4, pfxf]
    Bp = chp[:, 2:4]                      # [P, 2(wh), 4, pfxf]
    Cp = chp[:, 4]                        # [P, 4, pfxf]
    Mp = active[:npfx].astype(np.float32).reshape(P, pfxf)
    common = np.concatenate(
        [Ap.reshape(P, -1), Bp.reshape(P, -1), Cp.reshape(P, -1), Mp], axis=1)

    # per-core dense (p4,p9)/(t4,t9) replica of the LAST chunk's cells
    cl = CHUNK_CELLS[-1]
    a0 = CELLS_PP - cl
    lastx = xs[:, :, a0:, :][:, :, :, [4, 9]].reshape(NCORES, P, 2 * cl)
    lastt = ys[:, :, a0:, :][:, :, :, [4, 9]].reshape(NCORES, P, 2 * cl)
    return [
        {"xy": xyf[c],
         "pfx": np.ascontiguousarray(
             np.concatenate([common, lastx[c], lastt[c]], axis=1))}
        for c in range(NCORES)
    ], pfxf


def reduce_outputs(outs):
    """Per-core {"out": [128,3]} results -> scalar loss."""
    noobj = sum(
        o["out"][:, 0].astype(np.float64).sum() + o["out"][:, 2].astype(np.float64).sum()
        for o in outs
    )
    bbox = outs[0]["out"][:, 1].astype(np.float64).sum()
    return np.float32(L_NOOBJ * noobj + bbox)


_NC_CACHE = {}


def _get_nc(pfxf):
    if pfxf not in _NC_CACHE:
        _NC_CACHE[pfxf] = build_nc(pfxf=pfxf)
    return _NC_CACHE[pfxf]


def run(pred, target, **spmd_kwargs):
    in_maps, pfxf = make_inputs(pred, target)
    nc = _get_nc(pfxf)
    res = run_bass_kernel_spmd(nc, in_maps, list(range(NCORES)), **spmd_kwargs)
    return reduce_outputs(res.results), res


def kernel(pred, target):
    val, _ = run(pred, target)
    return val
